# revision 4
# baseline (speedup 1.0000x reference)
"""GCNConv Trainium2 kernel: out = segment_sum(edge_vals * (x @ W)[edge_row], edge_col) + bias.

Strategy (8-core SPMD):
  - Destination-node sharding (graph partitioning): the node range is split
    into 8 contiguous shards of 12544 nodes; each core receives the edges
    whose destination falls in its shard, the full node matrix x, and the
    replicated 128x128 weight. Output = concat of per-core shards.
  - Aggregate-then-project: aggr = segment_sum(val * x[row]); out = aggr @ W + b.
  - Within a core, destinations are processed in 64-node dest tiles. Edges are
    bucketed by (dest tile, source window) on the host (the sharding step) and
    padded to 128-edge tiles so all 8 cores run one identical program.
  - Source rows are fetched with the MoE dma_gather instruction (int16 indices,
    so x is addressed through <=32768-row windows). A 128-edge tile lands as
    [128 partitions(edge) x 128 feat] in SBUF.
  - The vector engine builds a val-scaled one-hot per edge tile in one
    tensor_scalar op: OV[e, j] = (iota[j] == colrel[e]) * val[e].
  - The tensor engine computes psum[feat, j] += Xg^T @ OV, accumulating all of
    a dest tile's edge tiles in PSUM (this performs the segment reduction,
    including duplicate destinations). A second matmul applies W; a K=1
    matmul of ones^T @ bias adds the bias; result DMAs out.
"""

import sys

for _p in ("/root/.axon_site/_ro/trn_rl_repo", "/opt/trn_rl_repo"):
    if _p not in sys.path:
        sys.path.append(_p)

from contextlib import ExitStack

import numpy as np

P = 128        # edge-tile size / SBUF partitions
DF = 128       # input feature dim
DO = 128       # output feature dim
W_TILE = 64    # dest-tile width (nodes per dest tile)
N_CORES = 8
GROUP = 8      # dest tiles per gather batch
WIN = 32768    # source window (int16 index limit)
N_QUEUES = 4   # SWDGE queues to rotate gather descriptor generation over
MAX_CALL_TILES = 8   # max 128-edge tiles per dma_gather call (1024 idxs; single-packet limit)
SINGLE_PACKET = True

# full-problem constants (kernel.py must be self-contained)
N_NODES = 100000
TILES_PER_CORE = 196              # 196 * 64 = 12544 nodes per core
NPC = TILES_PER_CORE * W_TILE


def _preprocess(edge_row, edge_col, edge_vals, n_nodes, tiles_per_core):
    """Bucket edges by (core, dest-tile, source-window); pad each bucket to a
    multiple of 128 with counts uniform across cores (SPMD requirement).

    Returns per-core SBUF-layout arrays:
      idx16  [128, T_core*8] int16  gather indices (window-relative), wrapped
                                    in 16 partitions and replicated 8x
      colrel [128, T_core] f32      dest slot within dest tile (-1 = dummy)
      val    [128, T_core] f32      edge weight (0 = dummy)
    plus the segment table seg[d][w] = (tile_start, n_tiles).
    """
    npc = tiles_per_core * W_TILE
    n_win = -(-n_nodes // WIN)
    n_gt = N_CORES * tiles_per_core

    row = np.asarray(edge_row, np.int64)
    col = np.asarray(edge_col, np.int64)
    v = np.asarray(edge_vals, np.float32)

    gt = col // W_TILE                      # global dest tile
    w = row // WIN                          # source window
    key = gt * n_win + w
    order = np.argsort(key)
    row, col, v, gt, w, key = row[order], col[order], v[order], gt[order], w[order], key[order]

    cnt = np.bincount(key, minlength=n_gt * n_win)          # [gt * n_win]
    seg_sorted_start = np.concatenate([[0], np.cumsum(cnt)])
    cnt_cdw = cnt.reshape(N_CORES, tiles_per_core, n_win)
    T_dw = (-(-cnt_cdw // P)).max(axis=0)                   # [tiles_per_core, n_win]

    # slot layout: groups of GROUP dest tiles; within a group, window-major
    # (so each (group, window) gather call covers contiguous edge tiles)
    tile_start = np.zeros((tiles_per_core, n_win), np.int64)
    group_calls = []   # per group: list of (w, tile_start, n_tiles)
    t_acc = 0
    d = 0
    while d < tiles_per_core:
        d_end = min(d + GROUP, tiles_per_core)
        calls = []
        for wi in range(n_win):
            c_start = t_acc
            for dd in range(d, d_end):
                tile_start[dd, wi] = t_acc
                t_acc += int(T_dw[dd, wi])
            if t_acc > c_start:
                calls.append((wi, c_start, t_acc - c_start))
        group_calls.append((d, d_end, calls))
        d = d_end
    T_core = t_acc

    # place each edge at its slot: seg base + rank within its (c,d,w) bucket
    seg_id = np.repeat(np.arange(n_gt * n_win), cnt)
    rank = np.arange(len(key)) - seg_sorted_start[seg_id]
    core = gt // tiles_per_core
    d_loc = gt % tiles_per_core
    slot = tile_start[d_loc, w] * P + rank                   # slot within its core

    flat_idx = np.zeros((N_CORES, T_core * P), np.int16)
    flat_col = np.full((N_CORES, T_core * P), -1.0, np.float32)
    flat_val = np.zeros((N_CORES, T_core * P), np.float32)
    flat_idx[core, slot] = (row - w * WIN).astype(np.int16)
    flat_col[core, slot] = (col - core * npc - d_loc * W_TILE).astype(np.float32)
    flat_val[core, slot] = v

    # [T*P] stream -> [P, T]: edge (tile t, slot p) = stream[t*P + p]
    colrel = np.ascontiguousarray(flat_col.reshape(N_CORES, T_core, P).transpose(0, 2, 1))
    val = np.ascontiguousarray(flat_val.reshape(N_CORES, T_core, P).transpose(0, 2, 1))
    # int16 wrap: [16, T*8] with blk[k%16, k//16] = flat[k]; replicate to 128 partitions
    blk = flat_idx.reshape(N_CORES, T_core * 8, 16).transpose(0, 2, 1)
    idx16 = np.ascontiguousarray(np.tile(blk, (1, 8, 1)))

    win_sizes = [min(WIN, n_nodes - wi * WIN) for wi in range(n_win)]
    return idx16, colrel, val, T_dw, tile_start, group_calls, T_core, n_win, win_sizes


def _build_program(n_nodes_in, tiles_per_core, T_dw, tile_start, group_calls, T_core, n_win):
    import concourse.bass as bass
    import concourse.tile as tile
    from concourse import bacc, mybir

    f32 = mybir.dt.float32
    nc = bacc.Bacc(
        "TRN2",
        target_bir_lowering=False,
        debug=False,
        num_devices=N_CORES,
        num_swdge_queues=N_QUEUES,
    )

    x_ap = nc.dram_tensor("x", [n_nodes_in, DF], f32, kind="ExternalInput").ap()
    w_ap = nc.dram_tensor("w", [DF, DO], f32, kind="ExternalInput").ap()
    bias_ap = nc.dram_tensor("bias", [1, DO], f32, kind="ExternalInput").ap()
    iota_ap = nc.dram_tensor("iota", [P, W_TILE], f32, kind="ExternalInput").ap()
    ones_ap = nc.dram_tensor("ones", [1, W_TILE], f32, kind="ExternalInput").ap()
    idx_ap = nc.dram_tensor("idx16", [P, T_core * 8], mybir.dt.int16, kind="ExternalInput").ap()
    cols_ap = nc.dram_tensor("cols", [P, T_core], f32, kind="ExternalInput").ap()
    vals_ap = nc.dram_tensor("vals", [P, T_core], f32, kind="ExternalInput").ap()
    out_ap = nc.dram_tensor(
        "out", [tiles_per_core * W_TILE, DO], f32, kind="ExternalOutput"
    ).ap()

    qn = [0]

    with tile.TileContext(nc) as tc, ExitStack() as ctx:
        consts = ctx.enter_context(tc.tile_pool(name="consts", bufs=1))
        gpool = ctx.enter_context(tc.tile_pool(name="gather", bufs=2))
        ovpool = ctx.enter_context(tc.tile_pool(name="ov", bufs=8))
        spool = ctx.enter_context(tc.tile_pool(name="stage", bufs=3))
        psum1 = ctx.enter_context(tc.tile_pool(name="psum1", bufs=2, space="PSUM"))
        psum2 = ctx.enter_context(tc.tile_pool(name="psum2", bufs=2, space="PSUM"))

        w_sb = consts.tile([DF, DO], f32, tag="w")
        nc.sync.dma_start(out=w_sb[:], in_=w_ap[:, :])
        bias_sb = consts.tile([1, DO], f32, tag="bias")
        nc.sync.dma_start(out=bias_sb[:], in_=bias_ap[:, :])
        iota_sb = consts.tile([P, W_TILE], f32, tag="iota")
        nc.sync.dma_start(out=iota_sb[:], in_=iota_ap[:, :])
        ones_sb = consts.tile([1, W_TILE], f32, tag="ones")
        nc.sync.dma_start(out=ones_sb[:], in_=ones_ap[:, :])
        idx_sb = consts.tile([P, T_core * 8], mybir.dt.int16, tag="idx16")
        nc.sync.dma_start(out=idx_sb[:], in_=idx_ap[:, :])
        cols_sb = consts.tile([P, T_core], f32, tag="cols")
        nc.sync.dma_start(out=cols_sb[:], in_=cols_ap[:, :])
        vals_sb = consts.tile([P, T_core], f32, tag="vals")
        nc.sync.dma_start(out=vals_sb[:], in_=vals_ap[:, :])

        for d0, d_end, calls in group_calls:
            g_t0 = int(calls[0][1])                      # first tile of group
            g_tn = int(calls[-1][1] + calls[-1][2])      # end tile
            n_t = g_tn - g_t0
            xg = gpool.tile([P, n_t * DF], f32, tag="xg")
            for wi, c_t0, c_nt in calls:
                win_lo = wi * WIN
                win_hi = min(win_lo + WIN, n_nodes_in)
                for s_t0 in range(c_t0, c_t0 + c_nt, MAX_CALL_TILES):
                    s_nt = min(MAX_CALL_TILES, c_t0 + c_nt - s_t0)
                    n_idx = s_nt * P
                    nc.gpsimd.dma_gather(
                        out_ap=xg[
                            :, (s_t0 - g_t0) * DF : (s_t0 - g_t0 + s_nt) * DF
                        ].rearrange("p (t f) -> p t f", f=DF),
                        in_ap=x_ap[win_lo:win_hi, :],
                        idxs_ap=idx_sb[:, s_t0 * 8 : (s_t0 + s_nt) * 8],
                        num_idxs=n_idx,
                        num_idxs_reg=n_idx,
                        elem_size=DF,
                        queue_num=qn[0],
                        single_packet=SINGLE_PACKET,
                    )
                    qn[0] = (qn[0] + 1) % N_QUEUES
            for dd in range(d0, d_end):
                tlist = []
                for wi in range(n_win):
                    ts, ntk = int(tile_start[dd, wi]), int(T_dw[dd, wi])
                    tlist.extend(range(ts, ts + ntk))
                if not tlist:
                    continue
                pt = psum1.tile([DF, W_TILE], f32, tag="aggrT")
                for i, t in enumerate(tlist):
                    ov = ovpool.tile([P, W_TILE], f32, tag="ov")
                    nc.vector.tensor_scalar(
                        out=ov[:],
                        in0=iota_sb[:],
                        scalar1=cols_sb[:, t : t + 1],
                        scalar2=vals_sb[:, t : t + 1],
                        op0=mybir.AluOpType.is_equal,
                        op1=mybir.AluOpType.mult,
                    )
                    nc.tensor.matmul(
                        out=pt[:],
                        lhsT=xg[:, (t - g_t0) * DF : (t - g_t0 + 1) * DF],
                        rhs=ov[:],
                        start=(i == 0),
                        stop=(i == len(tlist) - 1),
                    )
                aggrT = spool.tile([DF, W_TILE], f32, tag="aggrT_sb")
                nc.scalar.copy(out=aggrT[:], in_=pt[:])
                po = psum2.tile([W_TILE, DO], f32, tag="po")
                nc.tensor.matmul(
                    out=po[:], lhsT=aggrT[:], rhs=w_sb[:], start=True, stop=False
                )
                nc.tensor.matmul(
                    out=po[:], lhsT=ones_sb[:], rhs=bias_sb[:], start=False, stop=True
                )
                outt = spool.tile([W_TILE, DO], f32, tag="out_sb")
                nc.scalar.copy(out=outt[:], in_=po[:])
                nc.sync.dma_start(
                    out=out_ap[dd * W_TILE : (dd + 1) * W_TILE, :], in_=outt[:]
                )

    nc.compile()
    return nc


def _make_in_maps(x, weight1, bias1, idx16, colrel, val):
    iota = np.tile(np.arange(W_TILE, dtype=np.float32), (P, 1))
    ones = np.ones((1, W_TILE), np.float32)
    x = np.ascontiguousarray(np.asarray(x, np.float32))
    w = np.ascontiguousarray(np.asarray(weight1, np.float32))
    b = np.ascontiguousarray(np.asarray(bias1, np.float32).reshape(1, DO))
    return [
        {
            "x": x,
            "w": w,
            "bias": b,
            "iota": iota,
            "ones": ones,
            "idx16": idx16[c],
            "cols": colrel[c],
            "vals": val[c],
        }
        for c in range(N_CORES)
    ]


_CACHE = {}


def _prepare(x, edge_vals, weight1, bias1, edge_row, edge_col):
    idx16, colrel, val, T_dw, tile_start, group_calls, T_core, n_win, _ = _preprocess(
        edge_row, edge_col, edge_vals, N_NODES, TILES_PER_CORE
    )
    prog_key = ("prog", T_core, T_dw.tobytes())
    nc = _CACHE.get(prog_key)
    if nc is None:
        nc = _build_program(
            N_NODES, TILES_PER_CORE, T_dw, tile_start, group_calls, T_core, n_win
        )
        _CACHE.clear()
        _CACHE[prog_key] = nc
    in_maps = _make_in_maps(x, weight1, bias1, idx16, colrel, val)
    return nc, in_maps


def kernel(x, edge_vals, weight1, bias1, edge_row, edge_col):
    from concourse import bass_utils

    nc, in_maps = _prepare(
        np.asarray(x),
        np.asarray(edge_vals),
        np.asarray(weight1),
        np.asarray(bias1),
        np.asarray(edge_row),
        np.asarray(edge_col),
    )
    res = bass_utils.run_bass_kernel_spmd(nc, in_maps, core_ids=list(range(N_CORES)))
    full = np.concatenate([res.results[c]["out"] for c in range(N_CORES)], axis=0)
    return np.ascontiguousarray(full[:N_NODES])


# revision 6
# speedup vs baseline: 38.8992x; 38.8992x over previous
"""GCNConv Trainium2 kernel: out = segment_sum(edge_vals * (x @ W)[edge_row], edge_col) + bias.

Strategy (8-core SPMD):
  - Destination-node sharding (graph partitioning): the node range is split
    into 8 contiguous shards of 12544 nodes; each core receives the edges
    whose destination falls in its shard, the full node matrix x, and the
    replicated 128x128 weight. Output = concat of per-core shards.
  - Aggregate-then-project: aggr = segment_sum(val * x[row]); out = aggr @ W + b.
  - Within a core, destinations are processed in 64-node dest tiles. Edges are
    bucketed by (dest tile, source window) on the host (the sharding step) and
    padded to 128-edge tiles so all 8 cores run one identical program.
  - Source rows are fetched with the MoE dma_gather instruction (int16 indices,
    so x is addressed through <=32768-row windows). A 128-edge tile lands as
    [128 partitions(edge) x 128 feat] in SBUF.
  - The vector engine builds a val-scaled one-hot per edge tile in one
    tensor_scalar op: OV[e, j] = (iota[j] == colrel[e]) * val[e].
  - The tensor engine computes psum[feat, j] += Xg^T @ OV, accumulating all of
    a dest tile's edge tiles in PSUM (this performs the segment reduction,
    including duplicate destinations). A second matmul applies W; a K=1
    matmul of ones^T @ bias adds the bias; result DMAs out.
"""

import sys

for _p in ("/root/.axon_site/_ro/trn_rl_repo", "/opt/trn_rl_repo"):
    if _p not in sys.path:
        sys.path.append(_p)

from contextlib import ExitStack

import numpy as np

P = 128        # edge-tile size / SBUF partitions
DF = 128       # input feature dim
DO = 128       # output feature dim
W_TILE = 64    # dest-tile width (nodes per dest tile)
N_CORES = 8
GROUP = 8      # dest tiles per gather batch
WIN = 32768    # source window (int16 index limit)
N_QUEUES = 4   # SWDGE queues to rotate gather descriptor generation over
MAX_CALL_TILES = 8   # max 128-edge tiles per dma_gather call (1024 idxs; single-packet limit)
SINGLE_PACKET = True
REPEAT = 1     # repeat the kernel body (timing use only)

# full-problem constants (kernel.py must be self-contained)
N_NODES = 100000
TILES_PER_CORE = 196              # 196 * 64 = 12544 nodes per core
NPC = TILES_PER_CORE * W_TILE


def _preprocess(edge_row, edge_col, edge_vals, n_nodes, tiles_per_core):
    """Bucket edges by (core, dest-tile, source-window); pad each bucket to a
    multiple of 128 with counts uniform across cores (SPMD requirement).

    Returns per-core SBUF-layout arrays:
      idx16  [128, T_core*8] int16  gather indices (window-relative), wrapped
                                    in 16 partitions and replicated 8x
      colrel [128, T_core] f32      dest slot within dest tile (-1 = dummy)
      val    [128, T_core] f32      edge weight (0 = dummy)
    plus the segment table seg[d][w] = (tile_start, n_tiles).
    """
    npc = tiles_per_core * W_TILE
    n_win = -(-n_nodes // WIN)
    n_gt = N_CORES * tiles_per_core

    row = np.asarray(edge_row, np.int64)
    col = np.asarray(edge_col, np.int64)
    v = np.asarray(edge_vals, np.float32)

    gt = col // W_TILE                      # global dest tile
    w = row // WIN                          # source window
    key = gt * n_win + w
    order = np.argsort(key)
    row, col, v, gt, w, key = row[order], col[order], v[order], gt[order], w[order], key[order]

    cnt = np.bincount(key, minlength=n_gt * n_win)          # [gt * n_win]
    seg_sorted_start = np.concatenate([[0], np.cumsum(cnt)])
    cnt_cdw = cnt.reshape(N_CORES, tiles_per_core, n_win)
    T_dw = (-(-cnt_cdw // P)).max(axis=0)                   # [tiles_per_core, n_win]

    # slot layout: groups of GROUP dest tiles; within a group, window-major
    # (so each (group, window) gather call covers contiguous edge tiles)
    tile_start = np.zeros((tiles_per_core, n_win), np.int64)
    group_calls = []   # per group: list of (w, tile_start, n_tiles)
    t_acc = 0
    d = 0
    while d < tiles_per_core:
        d_end = min(d + GROUP, tiles_per_core)
        calls = []
        for wi in range(n_win):
            c_start = t_acc
            for dd in range(d, d_end):
                tile_start[dd, wi] = t_acc
                t_acc += int(T_dw[dd, wi])
            if t_acc > c_start:
                calls.append((wi, c_start, t_acc - c_start))
        group_calls.append((d, d_end, calls))
        d = d_end
    T_core = t_acc

    # place each edge at its slot: seg base + rank within its (c,d,w) bucket
    seg_id = np.repeat(np.arange(n_gt * n_win), cnt)
    rank = np.arange(len(key)) - seg_sorted_start[seg_id]
    core = gt // tiles_per_core
    d_loc = gt % tiles_per_core
    slot = tile_start[d_loc, w] * P + rank                   # slot within its core

    flat_idx = np.zeros((N_CORES, T_core * P), np.int16)
    flat_col = np.full((N_CORES, T_core * P), -1.0, np.float32)
    flat_val = np.zeros((N_CORES, T_core * P), np.float32)
    flat_idx[core, slot] = (row - w * WIN).astype(np.int16)
    flat_col[core, slot] = (col - core * npc - d_loc * W_TILE).astype(np.float32)
    flat_val[core, slot] = v

    # [T*P] stream -> [P, T]: edge (tile t, slot p) = stream[t*P + p]
    colrel = np.ascontiguousarray(flat_col.reshape(N_CORES, T_core, P).transpose(0, 2, 1))
    val = np.ascontiguousarray(flat_val.reshape(N_CORES, T_core, P).transpose(0, 2, 1))
    # int16 wrap: [16, T*8] with blk[k%16, k//16] = flat[k]; replicate to 128 partitions
    blk = flat_idx.reshape(N_CORES, T_core * 8, 16).transpose(0, 2, 1)
    idx16 = np.ascontiguousarray(np.tile(blk, (1, 8, 1)))

    win_sizes = [min(WIN, n_nodes - wi * WIN) for wi in range(n_win)]
    return idx16, colrel, val, T_dw, tile_start, group_calls, T_core, n_win, win_sizes


def _build_program(n_nodes_in, tiles_per_core, T_dw, tile_start, group_calls, T_core, n_win):
    import concourse.bass as bass
    import concourse.tile as tile
    from concourse import bacc, mybir

    f32 = mybir.dt.float32
    nc = bacc.Bacc(
        "TRN2",
        target_bir_lowering=False,
        debug=False,
        num_devices=N_CORES,
        num_swdge_queues=N_QUEUES,
    )

    x_ap = nc.dram_tensor("x", [n_nodes_in, DF], f32, kind="ExternalInput").ap()
    w_ap = nc.dram_tensor("w", [DF, DO], f32, kind="ExternalInput").ap()
    bias_ap = nc.dram_tensor("bias", [1, DO], f32, kind="ExternalInput").ap()
    iota_ap = nc.dram_tensor("iota", [P, W_TILE], f32, kind="ExternalInput").ap()
    ones_ap = nc.dram_tensor("ones", [1, W_TILE], f32, kind="ExternalInput").ap()
    idx_ap = nc.dram_tensor("idx16", [P, T_core * 8], mybir.dt.int16, kind="ExternalInput").ap()
    cols_ap = nc.dram_tensor("cols", [P, T_core], f32, kind="ExternalInput").ap()
    vals_ap = nc.dram_tensor("vals", [P, T_core], f32, kind="ExternalInput").ap()
    out_ap = nc.dram_tensor(
        "out", [tiles_per_core * W_TILE, DO], f32, kind="ExternalOutput"
    ).ap()

    qn = [0]

    with tile.TileContext(nc) as tc, ExitStack() as ctx:
        consts = ctx.enter_context(tc.tile_pool(name="consts", bufs=1))
        gpool = ctx.enter_context(tc.tile_pool(name="gather", bufs=2))
        ovpool = ctx.enter_context(tc.tile_pool(name="ov", bufs=8))
        spool = ctx.enter_context(tc.tile_pool(name="stage", bufs=3))
        psum1 = ctx.enter_context(tc.tile_pool(name="psum1", bufs=2, space="PSUM"))
        psum2 = ctx.enter_context(tc.tile_pool(name="psum2", bufs=2, space="PSUM"))

        w_sb = consts.tile([DF, DO], f32, tag="w")
        nc.sync.dma_start(out=w_sb[:], in_=w_ap[:, :])
        bias_sb = consts.tile([1, DO], f32, tag="bias")
        nc.sync.dma_start(out=bias_sb[:], in_=bias_ap[:, :])
        iota_sb = consts.tile([P, W_TILE], f32, tag="iota")
        nc.sync.dma_start(out=iota_sb[:], in_=iota_ap[:, :])
        ones_sb = consts.tile([1, W_TILE], f32, tag="ones")
        nc.sync.dma_start(out=ones_sb[:], in_=ones_ap[:, :])
        idx_sb = consts.tile([P, T_core * 8], mybir.dt.int16, tag="idx16")
        nc.sync.dma_start(out=idx_sb[:], in_=idx_ap[:, :])
        cols_sb = consts.tile([P, T_core], f32, tag="cols")
        nc.sync.dma_start(out=cols_sb[:], in_=cols_ap[:, :])
        vals_sb = consts.tile([P, T_core], f32, tag="vals")
        nc.sync.dma_start(out=vals_sb[:], in_=vals_ap[:, :])

        for d0, d_end, calls in group_calls * REPEAT:
            g_t0 = int(calls[0][1])                      # first tile of group
            g_tn = int(calls[-1][1] + calls[-1][2])      # end tile
            n_t = g_tn - g_t0
            xg = gpool.tile([P, n_t * DF], f32, tag="xg")
            for wi, c_t0, c_nt in calls:
                win_lo = wi * WIN
                win_hi = min(win_lo + WIN, n_nodes_in)
                for s_t0 in range(c_t0, c_t0 + c_nt, MAX_CALL_TILES):
                    s_nt = min(MAX_CALL_TILES, c_t0 + c_nt - s_t0)
                    n_idx = s_nt * P
                    nc.gpsimd.dma_gather(
                        out_ap=xg[
                            :, (s_t0 - g_t0) * DF : (s_t0 - g_t0 + s_nt) * DF
                        ].rearrange("p (t f) -> p t f", f=DF),
                        in_ap=x_ap[win_lo:win_hi, :],
                        idxs_ap=idx_sb[:, s_t0 * 8 : (s_t0 + s_nt) * 8],
                        num_idxs=n_idx,
                        num_idxs_reg=n_idx,
                        elem_size=DF,
                        queue_num=qn[0],
                        single_packet=SINGLE_PACKET,
                    )
                    qn[0] = (qn[0] + 1) % N_QUEUES
            for dd in range(d0, d_end):
                tlist = []
                for wi in range(n_win):
                    ts, ntk = int(tile_start[dd, wi]), int(T_dw[dd, wi])
                    tlist.extend(range(ts, ts + ntk))
                if not tlist:
                    continue
                pt = psum1.tile([DF, W_TILE], f32, tag="aggrT")
                for i, t in enumerate(tlist):
                    ov = ovpool.tile([P, W_TILE], f32, tag="ov")
                    nc.vector.tensor_scalar(
                        out=ov[:],
                        in0=iota_sb[:],
                        scalar1=cols_sb[:, t : t + 1],
                        scalar2=vals_sb[:, t : t + 1],
                        op0=mybir.AluOpType.is_equal,
                        op1=mybir.AluOpType.mult,
                    )
                    nc.tensor.matmul(
                        out=pt[:],
                        lhsT=xg[:, (t - g_t0) * DF : (t - g_t0 + 1) * DF],
                        rhs=ov[:],
                        start=(i == 0),
                        stop=(i == len(tlist) - 1),
                    )
                aggrT = spool.tile([DF, W_TILE], f32, tag="aggrT_sb")
                nc.scalar.copy(out=aggrT[:], in_=pt[:])
                po = psum2.tile([W_TILE, DO], f32, tag="po")
                nc.tensor.matmul(
                    out=po[:], lhsT=aggrT[:], rhs=w_sb[:], start=True, stop=False
                )
                nc.tensor.matmul(
                    out=po[:], lhsT=ones_sb[:], rhs=bias_sb[:], start=False, stop=True
                )
                outt = spool.tile([W_TILE, DO], f32, tag="out_sb")
                nc.scalar.copy(out=outt[:], in_=po[:])
                nc.sync.dma_start(
                    out=out_ap[dd * W_TILE : (dd + 1) * W_TILE, :], in_=outt[:]
                )

    nc.compile()
    return nc


def _make_in_maps(x, weight1, bias1, idx16, colrel, val):
    iota = np.tile(np.arange(W_TILE, dtype=np.float32), (P, 1))
    ones = np.ones((1, W_TILE), np.float32)
    x = np.ascontiguousarray(np.asarray(x, np.float32))
    w = np.ascontiguousarray(np.asarray(weight1, np.float32))
    b = np.ascontiguousarray(np.asarray(bias1, np.float32).reshape(1, DO))
    return [
        {
            "x": x,
            "w": w,
            "bias": b,
            "iota": iota,
            "ones": ones,
            "idx16": idx16[c],
            "cols": colrel[c],
            "vals": val[c],
        }
        for c in range(N_CORES)
    ]


_CACHE = {}


def _prepare(x, edge_vals, weight1, bias1, edge_row, edge_col):
    idx16, colrel, val, T_dw, tile_start, group_calls, T_core, n_win, _ = _preprocess(
        edge_row, edge_col, edge_vals, N_NODES, TILES_PER_CORE
    )
    prog_key = ("prog", T_core, T_dw.tobytes())
    nc = _CACHE.get(prog_key)
    if nc is None:
        nc = _build_program(
            N_NODES, TILES_PER_CORE, T_dw, tile_start, group_calls, T_core, n_win
        )
        _CACHE.clear()
        _CACHE[prog_key] = nc
    in_maps = _make_in_maps(x, weight1, bias1, idx16, colrel, val)
    return nc, in_maps


def kernel(x, edge_vals, weight1, bias1, edge_row, edge_col):
    from concourse import bass_utils

    nc, in_maps = _prepare(
        np.asarray(x),
        np.asarray(edge_vals),
        np.asarray(weight1),
        np.asarray(bias1),
        np.asarray(edge_row),
        np.asarray(edge_col),
    )
    res = bass_utils.run_bass_kernel_spmd(nc, in_maps, core_ids=list(range(N_CORES)))
    full = np.concatenate([res.results[c]["out"] for c in range(N_CORES)], axis=0)
    return np.ascontiguousarray(full[:N_NODES])
